# revision 14
# baseline (speedup 1.0000x reference)
"""AudioToMelSpectrogramPreprocessor on 8 TRN2 NeuronCores (Bass/Tile).

Per core (4 sequences, batch-sharded):
  pass1: pre-emphasis y = x - 0.97*x_prev (DVE), split y into yh=fp16(y) and
         yl=bf16(y-yh), write both to DRAM scratch with 160-sample reflect
         guards (reversals on DVE).
  pass2: per sequence: transposed strided-gather (DMA xbar transpose) of the
         overlapped frame matrix straight into [taps, frames] layout for both
         streams; DFT R/I = sum of 3 exact-product matmuls per K-chunk
         (yh@fp16(C) + yl@bf16(C) + yh@bf16(C-fp16(C))) accumulated in fp32
         PSUM -> ~2^-19 relative accuracy at fp16 speed (1 cyc/row);
         pow=R^2+I^2 (ACT squares + DVE add -> fp16), mel via fp16 matmul,
         log via ACT.
  stats: masked mean/var over valid frames (host-precomputed mask + 1/n
         scalars), normalize, zero padded frames, pad time dim to 1616.

Frequency bins 0..255 only: fb[:,0] and fb[:,256] are exactly zero.
Window support is taps [97,414]; we use [96,416) in chunks via three
128-wide transposed gathers at tap offsets {0,128,192} (xbar needs >=128
source columns), matmul K-chunks {128,64,128}.
"""
import numpy as np
import ml_dtypes

import concourse.bacc as bacc
import concourse.mybir as mybir
import concourse.tile as tile
from concourse.ap import AP
from concourse.bass_utils import run_bass_kernel_spmd

HOP = 160
NFFT = 512
NMEL = 64
PREEMPH = 0.97
LOG_GUARD = 2.0 ** (-24)
EPS = 1e-5
B, L = 32, 256000
NCORES = 8
BLOC = B // NCORES          # 4 sequences per core
T = L // HOP + 1            # 1601 frames
TPAD = 1616                 # padded to multiple of 16
NF = 256                    # freq bins 0..255
TAP0 = 96                   # window support [97,414] -> taps [96,416)
GOFF = [0, 128, 192]        # gather tap offsets (each 128 wide)
MK = [128, 64, 128]         # matmul K per chunk (chunk1 uses rows 0:64)
YLEN = 160 + L + 160        # 256320 (logical)
YALLOC = 1616 * HOP + 256   # 258816: gather reads 1616 frames (xbar needs
                            # row count %16==0); tail frames are never used
PCOLS = L // 128            # 2000
TT = [(0, 512), (512, 512), (1024, 512), (1536, T - 1536)]

F32 = mybir.dt.float32
F16 = mybir.dt.float16
BF16 = mybir.dt.bfloat16

_compiled = {}


def _build_bass():
    nc = bacc.Bacc("TRN2", target_bir_lowering=False)

    x_d = nc.dram_tensor("x", [BLOC, L], F32, kind="ExternalInput")
    mask_d = nc.dram_tensor("mask", [BLOC, T], F32, kind="ExternalInput")
    scal_d = nc.dram_tensor("scal", [BLOC, NMEL, 4], F32, kind="ExternalInput")
    ch_c = nc.dram_tensor("chc", [3, 128, NF], F16, kind="ExternalInput")
    ch_s = nc.dram_tensor("chs", [3, 128, NF], F16, kind="ExternalInput")
    cb_c = nc.dram_tensor("cbc", [3, 128, NF], BF16, kind="ExternalInput")
    cb_s = nc.dram_tensor("cbs", [3, 128, NF], BF16, kind="ExternalInput")
    cl_c = nc.dram_tensor("clc", [3, 128, NF], BF16, kind="ExternalInput")
    cl_s = nc.dram_tensor("cls", [3, 128, NF], BF16, kind="ExternalInput")
    fbt_d = nc.dram_tensor("fbt", [2, 128, NMEL], F16, kind="ExternalInput")
    out_d = nc.dram_tensor("out", [BLOC, NMEL, TPAD], F32, kind="ExternalOutput")

    with tile.TileContext(nc) as tc:
        with tc.tile_pool(name="const", bufs=1) as cpool, \
             tc.tile_pool(name="dram", bufs=1, space="DRAM") as dpool, \
             tc.tile_pool(name="p1", bufs=2) as p1, \
             tc.tile_pool(name="gat", bufs=2) as gat, \
             tc.tile_pool(name="pw", bufs=3) as pwp, \
             tc.tile_pool(name="seq", bufs=2) as seqp, \
             tc.tile_pool(name="st", bufs=1) as stp, \
             tc.tile_pool(name="psRI", bufs=2, space="PSUM") as psri, \
             tc.tile_pool(name="psM", bufs=2, space="PSUM") as psm:

            # constants
            def cload(name, d, dt):
                t = cpool.tile([128, 3, NF], dt, name=name)
                nc.sync.dma_start(out=t[:], in_=d[:].transpose([1, 0, 2]))
                return t
            chc = cload("chc_t", ch_c, F16)
            chs = cload("chs_t", ch_s, F16)
            cbc = cload("cbc_t", cb_c, BF16)
            cbs = cload("cbs_t", cb_s, BF16)
            clc = cload("clc_t", cl_c, BF16)
            cls = cload("cls_t", cl_s, BF16)
            fbt = cpool.tile([128, 2, NMEL], F16)
            nc.sync.dma_start(out=fbt[:], in_=fbt_d[:].transpose([1, 0, 2]))
            guard = cpool.tile([128, 1], F32)
            nc.vector.memset(guard[:], float(LOG_GUARD))

            yph = dpool.tile([BLOC, YALLOC], F16)
            ypl = dpool.tile([BLOC, YALLOC], BF16)

            for b in range(BLOC):
                # ---------------- pass 1 ----------------
                xt = p1.tile([128, PCOLS], F32, tag="xt")
                nc.sync.dma_start(out=xt[:], in_=x_d[b].rearrange("(p f) -> p f", p=128))
                xe = p1.tile([128, 1], F32, tag="xe")
                nc.vector.memset(xe[0:1, :], 0.0)
                nc.sync.dma_start(
                    out=xe[1:128, :],
                    in_=AP(x_d.ap().tensor, b * L + PCOLS - 1, [[PCOLS, 127], [1, 1]]))
                yt = p1.tile([128, PCOLS], F32, tag="yt")
                nc.vector.scalar_tensor_tensor(
                    out=yt[:, 1:PCOLS], in0=xt[:, 0:PCOLS - 1], scalar=-PREEMPH,
                    in1=xt[:, 1:PCOLS], op0=mybir.AluOpType.mult, op1=mybir.AluOpType.add)
                nc.vector.scalar_tensor_tensor(
                    out=yt[:, 0:1], in0=xe[:], scalar=-PREEMPH,
                    in1=xt[:, 0:1], op0=mybir.AluOpType.mult, op1=mybir.AluOpType.add)
                nc.vector.tensor_copy(yt[0:1, 0:1], xt[0:1, 0:1])

                # split: yh = fp16(y) (POOL cast), yl = bf16(y - yh) (DVE)
                yh = p1.tile([128, PCOLS], F16, tag="yh")
                nc.gpsimd.tensor_copy(yh[:], yt[:])
                yl = p1.tile([128, PCOLS], BF16, tag="yl")
                nc.vector.scalar_tensor_tensor(
                    out=yl[:], in0=yh[:], scalar=-1.0, in1=yt[:],
                    op0=mybir.AluOpType.mult, op1=mybir.AluOpType.add)

                for (stream, ypx, dt) in (("h", yph, F16), ("l", ypl, BF16)):
                    src = yh if stream == "h" else yl
                    nc.sync.dma_start(
                        out=ypx[b, 160:160 + L].rearrange("(p f) -> p f", p=128),
                        in_=src[:])
                    pitch = src.tensor.shape[1] * src.tensor.shape[2] \
                        if len(src.tensor.shape) > 2 else src.tensor.shape[1]
                    rvl = p1.tile([2, 160], dt, tag=f"rvl{stream}")
                    nc.vector.tensor_copy(
                        rvl[:], AP(src.tensor, 160, [[pitch, 2], [-1, 160]]))
                    nc.sync.dma_start(out=ypx[b, 0:160].unsqueeze(0), in_=rvl[0:1, :])
                    rvr = p1.tile([32, 160], dt, tag=f"rvr{stream}")
                    nc.vector.tensor_copy(
                        rvr[:], AP(src.tensor, 96 * pitch + (PCOLS - 2), [[pitch, 32], [-1, 160]]))
                    nc.sync.dma_start(out=ypx[b, 160 + L:YLEN].unsqueeze(0), in_=rvr[31:32, :])

                # ---------------- pass 2 ----------------
                gh, gl = [], []
                for c in range(3):
                    ght = gat.tile([128, TPAD], F16, tag=f"gh{c}")
                    nc.sync.dma_start_transpose(
                        ght[:], AP(yph.tensor, b * YALLOC + GOFF[c], [[HOP, TPAD], [1, 128]]))
                    gh.append(ght)
                    glt = gat.tile([128, TPAD], BF16, tag=f"gl{c}")
                    nc.sync.dma_start_transpose(
                        glt[:], AP(ypl.tensor, b * YALLOC + GOFF[c], [[HOP, TPAD], [1, 128]]))
                    gl.append(glt)

                pwf = seqp.tile([128, 2, T], F16, tag="pwf")
                melf = seqp.tile([NMEL, T], F32, tag="melf")
                logmel = seqp.tile([NMEL, T], F32, tag="logmel")
                for (t0, N) in TT:
                    for m in range(2):
                        fs = slice(128 * m, 128 * m + 128)
                        rr = psri.tile([128, N], F32, tag="rr")
                        ii = psri.tile([128, N], F32, tag="ii")
                        for (out_ps, ch, cb, cl) in ((rr, chc, cbc, clc),
                                                     (ii, chs, cbs, cls)):
                            k = 0
                            for (coef, gsrc) in ((ch, gh), (cb, gl), (cl, gh)):
                                for c in range(3):
                                    nc.tensor.matmul(
                                        out_ps[:], coef[0:MK[c], c, fs],
                                        gsrc[c][0:MK[c], t0:t0 + N],
                                        start=(k == 0), stop=(k == 8))
                                    k += 1
                        sq = pwp.tile([128, N], F32, tag="sq")
                        nc.scalar.square(sq[:], rr[:])
                        sqi = pwp.tile([128, N], F32, tag="sqi")
                        nc.scalar.square(sqi[:], ii[:])
                        nc.gpsimd.tensor_tensor(
                            pwf[:, m, t0:t0 + N], sq[:], sqi[:], mybir.AluOpType.add)
                    mel = psm.tile([NMEL, N], F32, tag="mel")
                    nc.tensor.matmul(mel[:], fbt[:, 0, :], pwf[:, 0, t0:t0 + N],
                                     start=True, stop=False)
                    nc.tensor.matmul(mel[:], fbt[:, 1, :], pwf[:, 1, t0:t0 + N],
                                     start=False, stop=True)
                    nc.vector.tensor_copy(melf[:, t0:t0 + N], mel[:])
                # single Ln per sequence (avoids ACT func-table thrash)
                nc.scalar.activation(
                    logmel[:], melf[:],
                    mybir.ActivationFunctionType.Ln, bias=guard[0:NMEL, 0:1])

                # ---------------- stats + normalize ----------------
                maskr = stp.tile([NMEL, T], F32, tag="maskr")
                nc.sync.dma_start(
                    out=maskr[:], in_=AP(mask_d.ap().tensor, b * T, [[0, NMEL], [1, T]]))
                scl = stp.tile([NMEL, 4], F32, tag="scl")
                nc.sync.dma_start(out=scl[:], in_=scal_d[b])

                zeroed = stp.tile([NMEL, T], F32, tag="zeroed")
                s1 = stp.tile([NMEL, 1], F32, tag="s1")
                nc.vector.scalar_tensor_tensor(
                    out=zeroed[:], in0=logmel[:], scalar=1.0, in1=maskr[:],
                    op0=mybir.AluOpType.bypass, op1=mybir.AluOpType.mult,
                    accum_out=s1[:])
                scratch = stp.tile([NMEL, T], F32, tag="scratch")
                s2 = stp.tile([NMEL, 1], F32, tag="s2")
                nc.scalar.activation(
                    scratch[:], zeroed[:], mybir.ActivationFunctionType.Square,
                    accum_out=s2[:])
                mu = stp.tile([NMEL, 1], F32, tag="mu")
                nc.vector.tensor_scalar_mul(mu[:], s1[:], scl[:, 0:1])
                m2 = stp.tile([NMEL, 1], F32, tag="m2")
                nc.vector.tensor_tensor(m2[:], s1[:], mu[:], mybir.AluOpType.mult)
                var = stp.tile([NMEL, 1], F32, tag="var")
                nc.vector.scalar_tensor_tensor(
                    out=var[:], in0=s2[:], scalar=1.0, in1=m2[:],
                    op0=mybir.AluOpType.bypass, op1=mybir.AluOpType.subtract)
                nc.vector.tensor_scalar_mul(var[:], var[:], scl[:, 1:2])
                sd = stp.tile([NMEL, 1], F32, tag="sd")
                nc.scalar.activation(sd[:], var[:], mybir.ActivationFunctionType.Sqrt)
                nc.vector.tensor_scalar_add(sd[:], sd[:], EPS)
                rstd = stp.tile([NMEL, 1], F32, tag="rstd")
                nc.vector.reciprocal(rstd[:], sd[:])
                mrs = stp.tile([NMEL, T], F32, tag="mrs")
                nc.vector.tensor_scalar_mul(mrs[:], maskr[:], rstd[:, 0:1])
                fin = stp.tile([NMEL, TPAD], F32, tag="fin")
                nc.vector.memset(fin[:, T:TPAD], 0.0)
                nc.vector.scalar_tensor_tensor(
                    out=fin[:, 0:T], in0=zeroed[:], scalar=mu[:, 0:1], in1=mrs[:],
                    op0=mybir.AluOpType.subtract, op1=mybir.AluOpType.mult)
                nc.sync.dma_start(out=out_d[b], in_=fin[:])

    nc.compile()
    return nc


def _host_prep(x, seq_len, window, fb):
    """Per-core input maps + feat_len."""
    x = np.ascontiguousarray(x, dtype=np.float32)
    seq_len = np.asarray(seq_len, dtype=np.int64)
    window = np.asarray(window, dtype=np.float32)
    fb = np.asarray(fb, dtype=np.float32)

    feat_len = (seq_len // HOP + 1).astype(np.int64)

    n = np.arange(NFFT, dtype=np.float64)
    f = np.arange(NF, dtype=np.float64)
    ang = 2.0 * np.pi * np.outer(n, f) / NFFT
    cosm = (window.astype(np.float64)[:, None] * np.cos(ang)).astype(np.float32)
    sinm = (window.astype(np.float64)[:, None] * np.sin(ang)).astype(np.float32)

    def split_chunks(cm):
        ch = np.zeros((3, 128, NF), dtype=np.float16)
        cb = np.zeros((3, 128, NF), dtype=ml_dtypes.bfloat16)
        cl = np.zeros((3, 128, NF), dtype=ml_dtypes.bfloat16)
        for c in range(3):
            rows = cm[TAP0 + GOFF[c]:TAP0 + GOFF[c] + MK[c]]
            h = rows.astype(np.float16)
            ch[c, :MK[c]] = h
            cb[c, :MK[c]] = rows.astype(ml_dtypes.bfloat16)
            cl[c, :MK[c]] = (rows - h.astype(np.float32)).astype(ml_dtypes.bfloat16)
        return ch, cb, cl

    chc, cbc, clc = split_chunks(cosm)
    chs, cbs, cls = split_chunks(sinm)

    fbt = np.zeros((2, 128, NMEL), dtype=np.float16)
    fbt[0] = fb[:, 0:128].T.astype(np.float16)
    fbt[1] = fb[:, 128:256].T.astype(np.float16)

    tgrid = np.arange(T)[None, :]
    in_maps = []
    for core in range(NCORES):
        sl = slice(core * BLOC, (core + 1) * BLOC)
        fl = feat_len[sl]
        mask = (tgrid < fl[:, None]).astype(np.float32)
        scal = np.zeros((BLOC, NMEL, 4), dtype=np.float32)
        scal[:, :, 0] = (1.0 / fl.astype(np.float64))[:, None]
        scal[:, :, 1] = (1.0 / (fl.astype(np.float64) - 1.0))[:, None]
        in_maps.append({
            "x": x[sl], "mask": mask, "scal": scal,
            "chc": chc, "chs": chs, "cbc": cbc, "cbs": cbs,
            "clc": clc, "cls": cls, "fbt": fbt,
        })
    return in_maps, feat_len


def kernel(x, seq_len, window, fb, _want_profile=False):
    if "nc" not in _compiled:
        _compiled["nc"] = _build_bass()
    nc = _compiled["nc"]

    in_maps, feat_len = _host_prep(x, seq_len, window, fb)
    res = run_bass_kernel_spmd(
        nc, in_maps, core_ids=list(range(NCORES)), trace=False)
    out = np.concatenate([r["out"] for r in res.results], axis=0)
    _compiled["last_exec_time_ns"] = res.exec_time_ns
    return out, feat_len.astype(np.int32)


# revision 17
# speedup vs baseline: 2.6145x; 2.6145x over previous
"""AudioToMelSpectrogramPreprocessor on 8 TRN2 NeuronCores (Bass/Tile).

Per core (4 sequences, batch-sharded):
  pass1: pre-emphasis y = x - 0.97*x_prev (DVE), split y into yh=fp16(y) and
         yl=bf16(y-yh), write both to DRAM scratch with 160-sample reflect
         guards (reversals on DVE).
  pass2: per sequence: transposed strided-gather (DMA xbar transpose) of the
         overlapped frame matrix straight into [taps, frames] layout for both
         streams; DFT R/I = sum of 3 exact-product matmuls per K-chunk
         (yh@fp16(C) + yl@bf16(C) + yh@bf16(C-fp16(C))) accumulated in fp32
         PSUM -> ~2^-19 relative accuracy at fp16 speed (1 cyc/row);
         pow=R^2+I^2 (ACT squares + DVE add -> fp16), mel via fp16 matmul,
         log via ACT.
  stats: masked mean/var over valid frames (host-precomputed mask + 1/n
         scalars), normalize, zero padded frames, pad time dim to 1616.

Frequency bins 0..255 only: fb[:,0] and fb[:,256] are exactly zero.
Window support is taps [97,414]; we use [96,416) in chunks via three
128-wide transposed gathers at tap offsets {0,128,192} (xbar needs >=128
source columns), matmul K-chunks {128,64,128}.
"""
import numpy as np
import ml_dtypes

import concourse.bacc as bacc
import concourse.mybir as mybir
import concourse.tile as tile
from concourse.ap import AP
from concourse.bass_utils import run_bass_kernel_spmd

HOP = 160
NFFT = 512
NMEL = 64
PREEMPH = 0.97
LOG_GUARD = 2.0 ** (-24)
EPS = 1e-5
B, L = 32, 256000
NCORES = 8
BLOC = B // NCORES          # 4 sequences per core
T = L // HOP + 1            # 1601 frames
TPAD = 1616                 # padded to multiple of 16
NF = 256                    # freq bins 0..255
TAP0 = 96                   # window support [97,414] -> taps [96,416)
GOFF = [0, 128, 192]        # gather tap offsets (each 128 wide)
MK = [128, 64, 128]         # matmul K per chunk (chunk1 uses rows 0:64)
YLEN = 160 + L + 160        # 256320 (logical)
YALLOC = 1616 * HOP + 256   # 258816: gather reads 1616 frames (xbar needs
                            # row count %16==0); tail frames are never used
PCOLS = L // 128            # 2000
TT = [(0, 512), (512, 512), (1024, 512), (1536, T - 1536)]

F32 = mybir.dt.float32
F16 = mybir.dt.float16
BF16 = mybir.dt.bfloat16

_compiled = {}


def _build_bass():
    nc = bacc.Bacc("TRN2", target_bir_lowering=False)

    x_d = nc.dram_tensor("x", [BLOC, L], F32, kind="ExternalInput")
    mask_d = nc.dram_tensor("mask", [BLOC, T], F32, kind="ExternalInput")
    scal_d = nc.dram_tensor("scal", [BLOC, NMEL, 4], F32, kind="ExternalInput")
    ch_c = nc.dram_tensor("chc", [3, 128, NF], F16, kind="ExternalInput")
    ch_s = nc.dram_tensor("chs", [3, 128, NF], F16, kind="ExternalInput")
    cb_c = nc.dram_tensor("cbc", [3, 128, NF], BF16, kind="ExternalInput")
    cb_s = nc.dram_tensor("cbs", [3, 128, NF], BF16, kind="ExternalInput")
    cl_c = nc.dram_tensor("clc", [3, 128, NF], BF16, kind="ExternalInput")
    cl_s = nc.dram_tensor("cls", [3, 128, NF], BF16, kind="ExternalInput")
    fbt_d = nc.dram_tensor("fbt", [2, 128, NMEL], F16, kind="ExternalInput")
    out_d = nc.dram_tensor("out", [BLOC, NMEL, TPAD], F32, kind="ExternalOutput")

    with tile.TileContext(nc) as tc:
        with tc.tile_pool(name="const", bufs=1) as cpool, \
             tc.tile_pool(name="dram", bufs=1, space="DRAM") as dpool, \
             tc.tile_pool(name="p1", bufs=2) as p1, \
             tc.tile_pool(name="gat", bufs=2) as gat, \
             tc.tile_pool(name="pw", bufs=3) as pwp, \
             tc.tile_pool(name="seq", bufs=2) as seqp, \
             tc.tile_pool(name="st", bufs=1) as stp, \
             tc.tile_pool(name="psRI", bufs=3, space="PSUM") as psri, \
             tc.tile_pool(name="psM", bufs=1, space="PSUM") as psm:

            # constants
            def cload(name, d, dt):
                t = cpool.tile([128, 3, NF], dt, name=name)
                nc.sync.dma_start(out=t[:], in_=d[:].transpose([1, 0, 2]))
                return t
            chc = cload("chc_t", ch_c, F16)
            chs = cload("chs_t", ch_s, F16)
            cbc = cload("cbc_t", cb_c, BF16)
            cbs = cload("cbs_t", cb_s, BF16)
            clc = cload("clc_t", cl_c, BF16)
            cls = cload("cls_t", cl_s, BF16)
            fbt = cpool.tile([128, 2, NMEL], F16)
            nc.sync.dma_start(out=fbt[:], in_=fbt_d[:].transpose([1, 0, 2]))
            guard = cpool.tile([128, 1], F32)
            nc.vector.memset(guard[:], float(LOG_GUARD))

            yph = dpool.tile([BLOC, YALLOC], F16)
            ypl = dpool.tile([BLOC, YALLOC], BF16)

            for b in range(BLOC):
                # ---------------- pass 1 ----------------
                xt = p1.tile([128, PCOLS], F32, tag="xt")
                nc.sync.dma_start(out=xt[:], in_=x_d[b].rearrange("(p f) -> p f", p=128))
                xe = p1.tile([128, 1], F32, tag="xe")
                nc.vector.memset(xe[0:1, :], 0.0)
                nc.sync.dma_start(
                    out=xe[1:128, :],
                    in_=AP(x_d.ap().tensor, b * L + PCOLS - 1, [[PCOLS, 127], [1, 1]]))
                yt = p1.tile([128, PCOLS], F32, tag="yt")
                nc.vector.scalar_tensor_tensor(
                    out=yt[:, 1:PCOLS], in0=xt[:, 0:PCOLS - 1], scalar=-PREEMPH,
                    in1=xt[:, 1:PCOLS], op0=mybir.AluOpType.mult, op1=mybir.AluOpType.add)
                nc.vector.scalar_tensor_tensor(
                    out=yt[:, 0:1], in0=xe[:], scalar=-PREEMPH,
                    in1=xt[:, 0:1], op0=mybir.AluOpType.mult, op1=mybir.AluOpType.add)
                nc.vector.tensor_copy(yt[0:1, 0:1], xt[0:1, 0:1])

                # split: yh = fp16(y) (POOL cast), yl = bf16(y - yh) (DVE)
                yh = p1.tile([128, PCOLS], F16, tag="yh")
                nc.gpsimd.tensor_copy(yh[:], yt[:])
                yl = p1.tile([128, PCOLS], BF16, tag="yl")
                nc.vector.scalar_tensor_tensor(
                    out=yl[:], in0=yh[:], scalar=-1.0, in1=yt[:],
                    op0=mybir.AluOpType.mult, op1=mybir.AluOpType.add)

                for (stream, ypx, dt) in (("h", yph, F16), ("l", ypl, BF16)):
                    src = yh if stream == "h" else yl
                    nc.sync.dma_start(
                        out=ypx[b, 160:160 + L].rearrange("(p f) -> p f", p=128),
                        in_=src[:])
                    pitch = src.tensor.shape[1] * src.tensor.shape[2] \
                        if len(src.tensor.shape) > 2 else src.tensor.shape[1]
                    rvl = p1.tile([2, 160], dt, tag=f"rvl{stream}")
                    nc.vector.tensor_copy(
                        rvl[:], AP(src.tensor, 160, [[pitch, 2], [-1, 160]]))
                    nc.sync.dma_start(out=ypx[b, 0:160].unsqueeze(0), in_=rvl[0:1, :])
                    rvr = p1.tile([32, 160], dt, tag=f"rvr{stream}")
                    nc.vector.tensor_copy(
                        rvr[:], AP(src.tensor, 96 * pitch + (PCOLS - 2), [[pitch, 32], [-1, 160]]))
                    nc.sync.dma_start(out=ypx[b, 160 + L:YLEN].unsqueeze(0), in_=rvr[31:32, :])

                # ---------------- pass 2 ----------------
                gh, gl = [], []
                for c in range(3):
                    ght = gat.tile([128, TPAD], F16, tag=f"gh{c}")
                    nc.sync.dma_start_transpose(
                        ght[:], AP(yph.tensor, b * YALLOC + GOFF[c], [[HOP, TPAD], [1, 128]]))
                    gh.append(ght)
                    glt = gat.tile([128, TPAD], BF16, tag=f"gl{c}")
                    nc.sync.dma_start_transpose(
                        glt[:], AP(ypl.tensor, b * YALLOC + GOFF[c], [[HOP, TPAD], [1, 128]]))
                    gl.append(glt)

                pwf = seqp.tile([128, 2, T], F16, tag="pwf")
                melf = seqp.tile([NMEL, T], F32, tag="melf")
                logmel = seqp.tile([NMEL, T], F32, tag="logmel")
                for (t0, N) in TT:
                    for m in range(2):
                        fs = slice(128 * m, 128 * m + 128)
                        rr = psri.tile([128, N], F32, tag="rr")
                        ii = psri.tile([128, N], F32, tag="ii")
                        for (out_ps, ch, cb, cl) in ((rr, chc, cbc, clc),
                                                     (ii, chs, cbs, cls)):
                            k = 0
                            for (coef, gsrc) in ((ch, gh), (cb, gl), (cl, gh)):
                                for c in range(3):
                                    nc.tensor.matmul(
                                        out_ps[:], coef[0:MK[c], c, fs],
                                        gsrc[c][0:MK[c], t0:t0 + N],
                                        start=(k == 0), stop=(k == 8))
                                    k += 1
                        sq = pwp.tile([128, N], F32, tag="sq")
                        nc.scalar.square(sq[:], rr[:])
                        sqi = pwp.tile([128, N], F32, tag="sqi")
                        nc.scalar.square(sqi[:], ii[:])
                        nc.gpsimd.tensor_tensor(
                            pwf[:, m, t0:t0 + N], sq[:], sqi[:], mybir.AluOpType.add)
                    mel = psm.tile([NMEL, N], F32, tag="mel")
                    nc.tensor.matmul(mel[:], fbt[:, 0, :], pwf[:, 0, t0:t0 + N],
                                     start=True, stop=False)
                    nc.tensor.matmul(mel[:], fbt[:, 1, :], pwf[:, 1, t0:t0 + N],
                                     start=False, stop=True)
                    nc.vector.tensor_copy(melf[:, t0:t0 + N], mel[:])
                # single Ln per sequence (avoids ACT func-table thrash)
                nc.scalar.activation(
                    logmel[:], melf[:],
                    mybir.ActivationFunctionType.Ln, bias=guard[0:NMEL, 0:1])

                # ---------------- stats + normalize ----------------
                maskr = stp.tile([NMEL, T], F32, tag="maskr")
                nc.sync.dma_start(
                    out=maskr[:], in_=AP(mask_d.ap().tensor, b * T, [[0, NMEL], [1, T]]))
                scl = stp.tile([NMEL, 4], F32, tag="scl")
                nc.sync.dma_start(out=scl[:], in_=scal_d[b])

                zeroed = stp.tile([NMEL, T], F32, tag="zeroed")
                s1 = stp.tile([NMEL, 1], F32, tag="s1")
                nc.vector.scalar_tensor_tensor(
                    out=zeroed[:], in0=logmel[:], scalar=1.0, in1=maskr[:],
                    op0=mybir.AluOpType.bypass, op1=mybir.AluOpType.mult,
                    accum_out=s1[:])
                scratch = stp.tile([NMEL, T], F32, tag="scratch")
                s2 = stp.tile([NMEL, 1], F32, tag="s2")
                nc.scalar.activation(
                    scratch[:], zeroed[:], mybir.ActivationFunctionType.Square,
                    accum_out=s2[:])
                mu = stp.tile([NMEL, 1], F32, tag="mu")
                nc.vector.tensor_scalar_mul(mu[:], s1[:], scl[:, 0:1])
                m2 = stp.tile([NMEL, 1], F32, tag="m2")
                nc.vector.tensor_tensor(m2[:], s1[:], mu[:], mybir.AluOpType.mult)
                var = stp.tile([NMEL, 1], F32, tag="var")
                nc.vector.scalar_tensor_tensor(
                    out=var[:], in0=s2[:], scalar=1.0, in1=m2[:],
                    op0=mybir.AluOpType.bypass, op1=mybir.AluOpType.subtract)
                nc.vector.tensor_scalar_mul(var[:], var[:], scl[:, 1:2])
                sd = stp.tile([NMEL, 1], F32, tag="sd")
                nc.scalar.activation(sd[:], var[:], mybir.ActivationFunctionType.Sqrt)
                nc.vector.tensor_scalar_add(sd[:], sd[:], EPS)
                rstd = stp.tile([NMEL, 1], F32, tag="rstd")
                nc.vector.reciprocal(rstd[:], sd[:])
                mrs = stp.tile([NMEL, T], F32, tag="mrs")
                nc.vector.tensor_scalar_mul(mrs[:], maskr[:], rstd[:, 0:1])
                fin = stp.tile([NMEL, TPAD], F32, tag="fin")
                nc.vector.memset(fin[:, T:TPAD], 0.0)
                nc.vector.scalar_tensor_tensor(
                    out=fin[:, 0:T], in0=zeroed[:], scalar=mu[:, 0:1], in1=mrs[:],
                    op0=mybir.AluOpType.subtract, op1=mybir.AluOpType.mult)
                nc.sync.dma_start(out=out_d[b], in_=fin[:])

    nc.compile()
    return nc


def _host_prep(x, seq_len, window, fb):
    """Per-core input maps + feat_len."""
    x = np.ascontiguousarray(x, dtype=np.float32)
    seq_len = np.asarray(seq_len, dtype=np.int64)
    window = np.asarray(window, dtype=np.float32)
    fb = np.asarray(fb, dtype=np.float32)

    feat_len = (seq_len // HOP + 1).astype(np.int64)

    n = np.arange(NFFT, dtype=np.float64)
    f = np.arange(NF, dtype=np.float64)
    ang = 2.0 * np.pi * np.outer(n, f) / NFFT
    cosm = (window.astype(np.float64)[:, None] * np.cos(ang)).astype(np.float32)
    sinm = (window.astype(np.float64)[:, None] * np.sin(ang)).astype(np.float32)

    def split_chunks(cm):
        ch = np.zeros((3, 128, NF), dtype=np.float16)
        cb = np.zeros((3, 128, NF), dtype=ml_dtypes.bfloat16)
        cl = np.zeros((3, 128, NF), dtype=ml_dtypes.bfloat16)
        for c in range(3):
            rows = cm[TAP0 + GOFF[c]:TAP0 + GOFF[c] + MK[c]]
            h = rows.astype(np.float16)
            ch[c, :MK[c]] = h
            cb[c, :MK[c]] = rows.astype(ml_dtypes.bfloat16)
            cl[c, :MK[c]] = (rows - h.astype(np.float32)).astype(ml_dtypes.bfloat16)
        return ch, cb, cl

    chc, cbc, clc = split_chunks(cosm)
    chs, cbs, cls = split_chunks(sinm)

    fbt = np.zeros((2, 128, NMEL), dtype=np.float16)
    fbt[0] = fb[:, 0:128].T.astype(np.float16)
    fbt[1] = fb[:, 128:256].T.astype(np.float16)

    tgrid = np.arange(T)[None, :]
    in_maps = []
    for core in range(NCORES):
        sl = slice(core * BLOC, (core + 1) * BLOC)
        fl = feat_len[sl]
        mask = (tgrid < fl[:, None]).astype(np.float32)
        scal = np.zeros((BLOC, NMEL, 4), dtype=np.float32)
        scal[:, :, 0] = (1.0 / fl.astype(np.float64))[:, None]
        scal[:, :, 1] = (1.0 / (fl.astype(np.float64) - 1.0))[:, None]
        in_maps.append({
            "x": x[sl], "mask": mask, "scal": scal,
            "chc": chc, "chs": chs, "cbc": cbc, "cbs": cbs,
            "clc": clc, "cls": cls, "fbt": fbt,
        })
    return in_maps, feat_len


def kernel(x, seq_len, window, fb, _want_profile=False):
    if "nc" not in _compiled:
        _compiled["nc"] = _build_bass()
    nc = _compiled["nc"]

    in_maps, feat_len = _host_prep(x, seq_len, window, fb)
    res = run_bass_kernel_spmd(
        nc, in_maps, core_ids=list(range(NCORES)), trace=False)
    out = np.concatenate([r["out"] for r in res.results], axis=0)
    _compiled["last_exec_time_ns"] = res.exec_time_ns
    return out, feat_len.astype(np.int32)


# revision 19
# speedup vs baseline: 3.9771x; 1.5212x over previous
"""AudioToMelSpectrogramPreprocessor on 8 TRN2 NeuronCores (Bass/Tile).

Per core (4 sequences, batch-sharded):
  pass1: pre-emphasis y = x - 0.97*x_prev (DVE), split y into yh=fp16(y) and
         yl=bf16(y-yh), write both to DRAM scratch with 160-sample reflect
         guards (reversals on DVE).
  pass2: per sequence: transposed strided-gather (DMA xbar transpose) of the
         overlapped frame matrix straight into [taps, frames] layout for both
         streams; DFT R/I = sum of 3 exact-product matmuls per K-chunk
         (yh@fp16(C) + yl@bf16(C) + yh@bf16(C-fp16(C))) accumulated in fp32
         PSUM -> ~2^-19 relative accuracy at fp16 speed (1 cyc/row);
         pow=R^2+I^2 (ACT squares + POOL add -> fp16 full-seq buffer), mel
         via fp16 matmul, one Ln per sequence on ACT (avoids LUT thrash).
  stats: masked mean/var over valid frames (host-precomputed mask + 1/n
         scalars), normalize, zero padded frames, pad time dim to 1616.

Frequency bins 0..255 only: fb[:,0] and fb[:,256] are exactly zero.
Window support is taps [97,414]; we use [96,416) in chunks via three
128-wide transposed gathers at tap offsets {0,128,192} (xbar needs >=128
source columns), matmul K-chunks {128,64,128}.
"""
import numpy as np
import ml_dtypes

import concourse.bacc as bacc
import concourse.mybir as mybir
import concourse.tile as tile
from concourse.ap import AP
from concourse.bass_utils import run_bass_kernel_spmd

HOP = 160
NFFT = 512
NMEL = 64
PREEMPH = 0.97
LOG_GUARD = 2.0 ** (-24)
EPS = 1e-5
B, L = 32, 256000
NCORES = 8
BLOC = B // NCORES          # 4 sequences per core
T = L // HOP + 1            # 1601 frames
TPAD = 1616                 # padded to multiple of 16
NF = 256                    # freq bins 0..255
TAP0 = 96                   # window support [97,414] -> taps [96,416)
GOFF = [0, 128, 192]        # gather tap offsets (each 128 wide)
MK = [128, 64, 128]         # matmul K per chunk (chunk1 uses rows 0:64)
YLEN = 160 + L + 160        # 256320 (logical)
YALLOC = 1616 * HOP + 256   # 258816: gather reads 1616 frames (xbar needs
                            # row count %16==0); tail frames are never used
PCOLS = L // 128            # 2000
TT = [(0, 512), (512, 512), (1024, 512), (1536, T - 1536)]

F32 = mybir.dt.float32
F16 = mybir.dt.float16
BF16 = mybir.dt.bfloat16

_compiled = {}


def _build_bass():
    nc = bacc.Bacc("TRN2", target_bir_lowering=False)

    x_d = nc.dram_tensor("x", [BLOC, L], F32, kind="ExternalInput")
    mask_d = nc.dram_tensor("mask", [BLOC, T], F32, kind="ExternalInput")
    scal_d = nc.dram_tensor("scal", [BLOC, NMEL, 4], F32, kind="ExternalInput")
    ch_c = nc.dram_tensor("chc", [3, 128, NF], F16, kind="ExternalInput")
    ch_s = nc.dram_tensor("chs", [3, 128, NF], F16, kind="ExternalInput")
    cb_c = nc.dram_tensor("cbc", [3, 128, NF], BF16, kind="ExternalInput")
    cb_s = nc.dram_tensor("cbs", [3, 128, NF], BF16, kind="ExternalInput")
    cl_c = nc.dram_tensor("clc", [3, 128, NF], BF16, kind="ExternalInput")
    cl_s = nc.dram_tensor("cls", [3, 128, NF], BF16, kind="ExternalInput")
    fbt_d = nc.dram_tensor("fbt", [2, 128, NMEL], F16, kind="ExternalInput")
    out_d = nc.dram_tensor("out", [BLOC, NMEL, TPAD], F32, kind="ExternalOutput")

    with tile.TileContext(nc) as tc:
        with tc.tile_pool(name="const", bufs=1) as cpool, \
             tc.tile_pool(name="dram", bufs=1, space="DRAM") as dpool, \
             tc.tile_pool(name="p1", bufs=2) as p1, \
             tc.tile_pool(name="gat", bufs=2) as gat, \
             tc.tile_pool(name="pw", bufs=3) as pwp, \
             tc.tile_pool(name="seq", bufs=2) as seqp, \
             tc.tile_pool(name="st", bufs=1) as stp, \
             tc.tile_pool(name="psRI", bufs=3, space="PSUM") as psri, \
             tc.tile_pool(name="psM", bufs=1, space="PSUM") as psm:

            # constants
            def cload(name, d, dt):
                t = cpool.tile([128, 3, NF], dt, name=name)
                nc.sync.dma_start(out=t[:], in_=d[:].transpose([1, 0, 2]))
                return t
            chc = cload("chc_t", ch_c, F16)
            chs = cload("chs_t", ch_s, F16)
            cbc = cload("cbc_t", cb_c, BF16)
            cbs = cload("cbs_t", cb_s, BF16)
            clc = cload("clc_t", cl_c, BF16)
            cls = cload("cls_t", cl_s, BF16)
            fbt = cpool.tile([128, 2, NMEL], F16)
            nc.sync.dma_start(out=fbt[:], in_=fbt_d[:].transpose([1, 0, 2]))
            guard = cpool.tile([128, 1], F32)
            nc.vector.memset(guard[:], float(LOG_GUARD))

            yph = dpool.tile([BLOC, YALLOC], F16)
            ypl = dpool.tile([BLOC, YALLOC], BF16)

            for b in range(BLOC):
                # ---------------- pass 1 ----------------
                xt = p1.tile([128, PCOLS], F32, tag="xt")
                nc.sync.dma_start(out=xt[:], in_=x_d[b].rearrange("(p f) -> p f", p=128))
                xe = p1.tile([128, 1], F32, tag="xe")
                nc.vector.memset(xe[0:1, :], 0.0)
                nc.sync.dma_start(
                    out=xe[1:128, :],
                    in_=AP(x_d.ap().tensor, b * L + PCOLS - 1, [[PCOLS, 127], [1, 1]]))
                yt = p1.tile([128, PCOLS], F32, tag="yt")
                nc.vector.scalar_tensor_tensor(
                    out=yt[:, 1:PCOLS], in0=xt[:, 0:PCOLS - 1], scalar=-PREEMPH,
                    in1=xt[:, 1:PCOLS], op0=mybir.AluOpType.mult, op1=mybir.AluOpType.add)
                nc.vector.scalar_tensor_tensor(
                    out=yt[:, 0:1], in0=xe[:], scalar=-PREEMPH,
                    in1=xt[:, 0:1], op0=mybir.AluOpType.mult, op1=mybir.AluOpType.add)
                nc.vector.tensor_copy(yt[0:1, 0:1], xt[0:1, 0:1])

                # split: yh = fp16(y) (POOL cast), yl = bf16(y - yh) (DVE)
                yh = p1.tile([128, PCOLS], F16, tag="yh")
                nc.gpsimd.tensor_copy(yh[:], yt[:])
                yl = p1.tile([128, PCOLS], BF16, tag="yl")
                nc.vector.scalar_tensor_tensor(
                    out=yl[:], in0=yh[:], scalar=-1.0, in1=yt[:],
                    op0=mybir.AluOpType.mult, op1=mybir.AluOpType.add)

                for (stream, ypx, dt) in (("h", yph, F16), ("l", ypl, BF16)):
                    src = yh if stream == "h" else yl
                    nc.sync.dma_start(
                        out=ypx[b, 160:160 + L].rearrange("(p f) -> p f", p=128),
                        in_=src[:])
                    pitch = src.tensor.shape[1] * src.tensor.shape[2] \
                        if len(src.tensor.shape) > 2 else src.tensor.shape[1]
                    rvl = p1.tile([2, 160], dt, tag=f"rvl{stream}")
                    nc.vector.tensor_copy(
                        rvl[:], AP(src.tensor, 160, [[pitch, 2], [-1, 160]]))
                    nc.sync.dma_start(out=ypx[b, 0:160].unsqueeze(0), in_=rvl[0:1, :])
                    rvr = p1.tile([32, 160], dt, tag=f"rvr{stream}")
                    nc.vector.tensor_copy(
                        rvr[:], AP(src.tensor, 96 * pitch + (PCOLS - 2), [[pitch, 32], [-1, 160]]))
                    nc.sync.dma_start(out=ypx[b, 160 + L:YLEN].unsqueeze(0), in_=rvr[31:32, :])

                # ---------------- pass 2 ----------------
                gh, gl = [], []
                for c in range(3):
                    ght = gat.tile([128, TPAD], F16, tag=f"gh{c}")
                    nc.sync.dma_start_transpose(
                        ght[:], AP(yph.tensor, b * YALLOC + GOFF[c], [[HOP, TPAD], [1, 128]]))
                    gh.append(ght)
                    glt = gat.tile([128, TPAD], BF16, tag=f"gl{c}")
                    nc.sync.dma_start_transpose(
                        glt[:], AP(ypl.tensor, b * YALLOC + GOFF[c], [[HOP, TPAD], [1, 128]]))
                    gl.append(glt)

                pwf = seqp.tile([128, 2, T], F16, tag="pwf")
                melf = seqp.tile([NMEL, T], F32, tag="melf")
                logmel = seqp.tile([NMEL, T], F32, tag="logmel")
                for (t0, N) in TT:
                    for m in range(2):
                        fs = slice(128 * m, 128 * m + 128)
                        rr = psri.tile([128, N], F32, tag="rr")
                        ii = psri.tile([128, N], F32, tag="ii")
                        for (out_ps, ch, cb, cl) in ((rr, chc, cbc, clc),
                                                     (ii, chs, cbs, cls)):
                            k = 0
                            for (coef, gsrc) in ((ch, gh), (cb, gl), (cl, gh)):
                                for c in range(3):
                                    nc.tensor.matmul(
                                        out_ps[:], coef[0:MK[c], c, fs],
                                        gsrc[c][0:MK[c], t0:t0 + N],
                                        start=(k == 0), stop=(k == 8))
                                    k += 1
                        sq = pwp.tile([128, N], F32, tag="sq")
                        nc.scalar.square(sq[:], rr[:])
                        sqi = pwp.tile([128, N], F32, tag="sqi")
                        nc.scalar.square(sqi[:], ii[:])
                        nc.gpsimd.tensor_tensor(
                            pwf[:, m, t0:t0 + N], sq[:], sqi[:], mybir.AluOpType.add)
                    mel = psm.tile([NMEL, N], F32, tag="mel")
                    nc.tensor.matmul(mel[:], fbt[:, 0, :], pwf[:, 0, t0:t0 + N],
                                     start=True, stop=False)
                    nc.tensor.matmul(mel[:], fbt[:, 1, :], pwf[:, 1, t0:t0 + N],
                                     start=False, stop=True)
                    nc.vector.tensor_copy(melf[:, t0:t0 + N], mel[:])
                # single Ln per sequence (avoids ACT func-table thrash)
                nc.scalar.activation(
                    logmel[:], melf[:],
                    mybir.ActivationFunctionType.Ln, bias=guard[0:NMEL, 0:1])

                # ---------------- stats + normalize ----------------
                maskr = stp.tile([NMEL, T], F32, tag="maskr")
                nc.sync.dma_start(
                    out=maskr[:], in_=AP(mask_d.ap().tensor, b * T, [[0, NMEL], [1, T]]))
                scl = stp.tile([NMEL, 4], F32, tag="scl")
                nc.sync.dma_start(out=scl[:], in_=scal_d[b])

                zeroed = stp.tile([NMEL, T], F32, tag="zeroed")
                s1 = stp.tile([NMEL, 1], F32, tag="s1")
                nc.vector.scalar_tensor_tensor(
                    out=zeroed[:], in0=logmel[:], scalar=1.0, in1=maskr[:],
                    op0=mybir.AluOpType.bypass, op1=mybir.AluOpType.mult,
                    accum_out=s1[:])
                scratch = stp.tile([NMEL, T], F32, tag="scratch")
                s2 = stp.tile([NMEL, 1], F32, tag="s2")
                nc.scalar.activation(
                    scratch[:], zeroed[:], mybir.ActivationFunctionType.Square,
                    accum_out=s2[:])
                mu = stp.tile([NMEL, 1], F32, tag="mu")
                nc.vector.tensor_scalar_mul(mu[:], s1[:], scl[:, 0:1])
                m2 = stp.tile([NMEL, 1], F32, tag="m2")
                nc.vector.tensor_tensor(m2[:], s1[:], mu[:], mybir.AluOpType.mult)
                var = stp.tile([NMEL, 1], F32, tag="var")
                nc.vector.scalar_tensor_tensor(
                    out=var[:], in0=s2[:], scalar=1.0, in1=m2[:],
                    op0=mybir.AluOpType.bypass, op1=mybir.AluOpType.subtract)
                nc.vector.tensor_scalar_mul(var[:], var[:], scl[:, 1:2])
                sd = stp.tile([NMEL, 1], F32, tag="sd")
                nc.scalar.activation(sd[:], var[:], mybir.ActivationFunctionType.Sqrt)
                nc.vector.tensor_scalar_add(sd[:], sd[:], EPS)
                rstd = stp.tile([NMEL, 1], F32, tag="rstd")
                nc.vector.reciprocal(rstd[:], sd[:])
                mrs = stp.tile([NMEL, T], F32, tag="mrs")
                nc.vector.tensor_scalar_mul(mrs[:], maskr[:], rstd[:, 0:1])
                fin = stp.tile([NMEL, TPAD], F32, tag="fin")
                nc.vector.memset(fin[:, T:TPAD], 0.0)
                nc.vector.scalar_tensor_tensor(
                    out=fin[:, 0:T], in0=zeroed[:], scalar=mu[:, 0:1], in1=mrs[:],
                    op0=mybir.AluOpType.subtract, op1=mybir.AluOpType.mult)
                nc.sync.dma_start(out=out_d[b], in_=fin[:])

    nc.compile()
    return nc


def _host_prep(x, seq_len, window, fb):
    """Per-core input maps + feat_len."""
    x = np.ascontiguousarray(x, dtype=np.float32)
    seq_len = np.asarray(seq_len, dtype=np.int64)
    window = np.asarray(window, dtype=np.float32)
    fb = np.asarray(fb, dtype=np.float32)

    feat_len = (seq_len // HOP + 1).astype(np.int64)

    n = np.arange(NFFT, dtype=np.float64)
    f = np.arange(NF, dtype=np.float64)
    ang = 2.0 * np.pi * np.outer(n, f) / NFFT
    cosm = (window.astype(np.float64)[:, None] * np.cos(ang)).astype(np.float32)
    sinm = (window.astype(np.float64)[:, None] * np.sin(ang)).astype(np.float32)

    def split_chunks(cm):
        ch = np.zeros((3, 128, NF), dtype=np.float16)
        cb = np.zeros((3, 128, NF), dtype=ml_dtypes.bfloat16)
        cl = np.zeros((3, 128, NF), dtype=ml_dtypes.bfloat16)
        for c in range(3):
            rows = cm[TAP0 + GOFF[c]:TAP0 + GOFF[c] + MK[c]]
            h = rows.astype(np.float16)
            ch[c, :MK[c]] = h
            cb[c, :MK[c]] = rows.astype(ml_dtypes.bfloat16)
            cl[c, :MK[c]] = (rows - h.astype(np.float32)).astype(ml_dtypes.bfloat16)
        return ch, cb, cl

    chc, cbc, clc = split_chunks(cosm)
    chs, cbs, cls = split_chunks(sinm)

    fbt = np.zeros((2, 128, NMEL), dtype=np.float16)
    fbt[0] = fb[:, 0:128].T.astype(np.float16)
    fbt[1] = fb[:, 128:256].T.astype(np.float16)

    tgrid = np.arange(T)[None, :]
    in_maps = []
    for core in range(NCORES):
        sl = slice(core * BLOC, (core + 1) * BLOC)
        fl = feat_len[sl]
        mask = (tgrid < fl[:, None]).astype(np.float32)
        scal = np.zeros((BLOC, NMEL, 4), dtype=np.float32)
        scal[:, :, 0] = (1.0 / fl.astype(np.float64))[:, None]
        scal[:, :, 1] = (1.0 / (fl.astype(np.float64) - 1.0))[:, None]
        in_maps.append({
            "x": x[sl], "mask": mask, "scal": scal,
            "chc": chc, "chs": chs, "cbc": cbc, "cbs": cbs,
            "clc": clc, "cls": cls, "fbt": fbt,
        })
    return in_maps, feat_len


def kernel(x, seq_len, window, fb, _want_profile=False):
    if "nc" not in _compiled:
        _compiled["nc"] = _build_bass()
    nc = _compiled["nc"]

    in_maps, feat_len = _host_prep(x, seq_len, window, fb)
    res = None
    for attempt in range(3):
        try:
            res = run_bass_kernel_spmd(
                nc, in_maps, core_ids=list(range(NCORES)), trace=False)
            break
        except Exception:
            if attempt == 2:
                raise
            import time
            time.sleep(20)
    out = np.concatenate([r["out"] for r in res.results], axis=0)
    _compiled["last_exec_time_ns"] = res.exec_time_ns
    return out, feat_len.astype(np.int32)
